# revision 36
# baseline (speedup 1.0000x reference)
"""AdaptiveLiquidNeuron forward on 8 TRN2 NeuronCores (data-parallel over batch).

Math (per batch row, H=1024):
  context = relu(h @ W1.T + b1) @ W2.T + b2
  pa      = context @ PM.T + pm_b
  mm      = (1 + pa) * (e @ Wrec.T)
  dh      = (-decay*h + mm + bias) / (tau * sigmoid(pa))
  out     = LayerNorm(dh) * ln_w + ln_b

Strategy: shard B=16384 over 8 cores (2048 rows each), replicate H x H weights.
On-chip everything is kept transposed ([H on partitions, B on free]) so the four
matmuls need no on-chip transposes (host pre-transposes weights + activations).
LayerNorm reduces over the partition axis via ones-matmuls (sum and sum-of-
squares side by side in one rhs); rstd = 2*Dsqrt(var+eps) with the 2 folded into
ln_w host-side; stats broadcast back across partitions with one K=1 matmul.
Host folds 1/tau into Wrec/decay/bias, ce_b2 into pm_b, and uses
1/sigmoid(x) = 1 + exp(-x).
"""

import numpy as np
import ml_dtypes

BF16 = ml_dtypes.bfloat16

B, H = 16384, 1024
NCORES = 8
BL = B // NCORES      # 2048 batch rows per core
P = 128               # partitions
KC = H // P           # 8 chunks of the hidden dim
NB = 8                # batch tiles per core
NT = BL // NB         # 256 batch columns per tile
EPS = 1e-5

# consts layout: [128, 6*KC] f32, column v*KC + m = chunk m of vector v
V_B1, V_PMB, V_NDEC, V_BIASP, V_LNW2, V_LNB = range(6)

_CACHED = {}


def _build_nc(lnb_zero):
    import concourse.bacc as bacc
    import concourse.tile as tile
    from concourse import mybir
    from contextlib import ExitStack

    f32 = mybir.dt.float32
    bf16 = mybir.dt.bfloat16
    AF = mybir.ActivationFunctionType
    OP = mybir.AluOpType

    nc = bacc.Bacc(target_bir_lowering=False)

    hT_e = nc.declare_dram_parameter("hT", [H, BL], bf16, isOutput=False)
    eT_e = nc.declare_dram_parameter("eT", [H, BL], bf16, isOutput=False)
    w1_e = nc.declare_dram_parameter("w1T", [H, H], bf16, isOutput=False)
    w2_e = nc.declare_dram_parameter("w2T", [H, H], bf16, isOutput=False)
    pm_e = nc.declare_dram_parameter("pmT", [H, H], bf16, isOutput=False)
    wr_e = nc.declare_dram_parameter("wrT", [H, H], bf16, isOutput=False)
    cs_e = nc.declare_dram_parameter("consts", [P, 6 * KC], f32, isOutput=False)
    out_e = nc.declare_dram_parameter("out", [H, BL], f32, isOutput=True)

    hT_r = hT_e[:].rearrange("(k p) b -> p k b", p=P)
    eT_r = eT_e[:].rearrange("(k p) b -> p k b", p=P)
    out_r = out_e[:].rearrange("(m p) b -> p m b", p=P)

    with tile.TileContext(nc) as tc, ExitStack() as ctx:
        wpool = ctx.enter_context(tc.tile_pool(name="weights", bufs=1))
        cpool = ctx.enter_context(tc.tile_pool(name="consts", bufs=1))
        iopool = ctx.enter_context(tc.tile_pool(name="io", bufs=3))
        actpool = ctx.enter_context(tc.tile_pool(name="acts", bufs=1))
        fpool = ctx.enter_context(tc.tile_pool(name="f32work", bufs=1))
        dhpool = ctx.enter_context(tc.tile_pool(name="dh", bufs=2))
        rpool = ctx.enter_context(tc.tile_pool(name="redu", bufs=2))
        ypool = ctx.enter_context(tc.tile_pool(name="y", bufs=4))
        rowpool = ctx.enter_context(tc.tile_pool(name="rows", bufs=2))
        outpool = ctx.enter_context(tc.tile_pool(name="outs", bufs=1))
        psA = ctx.enter_context(tc.tile_pool(name="psA", bufs=4, space="PSUM"))
        psS = ctx.enter_context(tc.tile_pool(name="psS", bufs=1, space="PSUM"))
        psB = ctx.enter_context(tc.tile_pool(name="psB", bufs=2, space="PSUM"))

        # ---- resident constants / weights ----
        # Prologue latency: mm1 needs consts+w1+hT0 first — split those
        # halves across the sync and gpsimd DMA queues so they stream in
        # parallel; everything else queues up behind in need-order.
        consts = cpool.tile([P, 6 * KC], f32, tag="consts")
        nc.sync.dma_start(out=consts[:], in_=cs_e[:])

        def col(v, m):
            return consts[:, v * KC + m : v * KC + m + 1]

        w_sb = {}
        for nm, ext in (("w1", w1_e), ("w2", w2_e), ("pm", pm_e), ("wr", wr_e)):
            w_sb[nm] = (wpool.tile([P, KC, H], bf16, tag=nm, name=f"w_{nm}"), ext)

        def load_w(nm, eng, lo=0, hi=KC):
            t, ext = w_sb[nm]
            src = ext[:].rearrange("(k p) m -> p k m", p=P)
            eng.dma_start(out=t[:, lo:hi, :], in_=src[:, lo:hi, :])
            return t

        def load_io(i, h_eng, e_eng, split=False):
            ht = iopool.tile([P, KC, NT], bf16, tag="hT")
            et = iopool.tile([P, KC, NT], bf16, tag="eT")
            bs = slice(i * NT, (i + 1) * NT)
            if split:
                h_eng.dma_start(out=ht[:, 0 : KC // 2, :],
                                in_=hT_r[:, 0 : KC // 2, bs])
                e_eng.dma_start(out=ht[:, KC // 2 :, :],
                                in_=hT_r[:, KC // 2 :, bs])
            else:
                h_eng.dma_start(out=ht[:], in_=hT_r[:, :, bs])
            e_eng.dma_start(out=et[:], in_=eT_r[:, :, bs])
            return ht, et

        # fan the critical prologue transfers out over three DMA queues
        w1_sb = load_w("w1", nc.sync, 0, 3)
        load_w("w1", nc.scalar, 3, 6)
        load_w("w1", nc.gpsimd, 6, 8)
        io_tiles = [load_io(0, nc.sync, nc.scalar, split=True), None]
        w2_sb = load_w("w2", nc.gpsimd)
        wr_sb = load_w("wr", nc.sync)
        pm_sb = load_w("pm", nc.scalar)
        io_tiles[1] = load_io(1, nc.gpsimd, nc.gpsimd)

        ones_col = cpool.tile([P, 1], bf16, tag="ones_col")
        nc.vector.memset(ones_col[:], 1.0)
        ones_row = cpool.tile([1, P], f32, tag="ones_row")
        nc.vector.memset(ones_row[:], 1.0)


        state = [None] * NB

        def mm_layer(w, rhs_t, evac):
            """psum[m] = w[:,:,m].T @ rhs (contract KC chunks); evac(m, psum)."""
            for m in range(KC):
                acc = psA.tile([P, NT], f32, tag="acc")
                for k in range(KC):
                    nc.tensor.matmul(
                        acc[:],
                        w[:, k, m * P : (m + 1) * P],
                        rhs_t[:, k, :],
                        start=(k == 0),
                        stop=(k == KC - 1),
                    )
                evac(m, acc)

        def matmul_phase(i, pe_hook1, pe_hook2):
            ht, et = io_tiles[i % 2]
            if i + 2 < NB:
                io_tiles[i % 2] = load_io(i + 2, nc.sync, nc.sync)

            c1 = actpool.tile([P, KC, NT], bf16, tag="c1")
            cx = actpool.tile([P, KC, NT], bf16, tag="ctx")
            pa = fpool.tile([P, KC, NT], f32, tag="pa")
            ex = fpool.tile([P, KC, NT], f32, tag="exp")
            t2 = fpool.tile([P, KC, NT], f32, tag="t2")
            u = fpool.tile([P, KC, NT], f32, tag="u")
            num = fpool.tile([P, KC, NT], f32, tag="num")
            dh = dhpool.tile([P, KC, NT], f32, tag="dh")
            sq = rpool.tile([P, KC, NT], bf16, tag="sq")

            # context encoder layer 1: c1 = relu(W1 @ hT + b1)
            mm_layer(
                w1_sb,
                ht,
                lambda m, acc: nc.scalar.activation(
                    c1[:, m, :], acc[:], AF.Relu, bias=col(V_B1, m), scale=1.0
                ),
            )
            pe_hook1()  # reductions of tile i-1 slot in here on PE
            # context encoder layer 2 (b2 folded into pm_b): ctx = W2 @ c1
            mm_layer(
                w2_sb,
                c1,
                lambda m, acc: nc.scalar.activation(
                    cx[:, m, :], acc[:], AF.Copy, bias=0.0, scale=1.0
                ),
            )
            pe_hook2()  # stat broadcast of tile i-1
            # param modulator: pa = PM @ ctx + pm_b'
            mm_layer(
                pm_sb,
                cx,
                lambda m, acc: nc.vector.tensor_scalar_add(
                    pa[:, m, :], acc[:], col(V_PMB, m)
                ),
            )
            # 1/sigmoid(pa) = 1 + exp(-pa)
            nc.scalar.activation(ex[:], pa[:], AF.Exp, bias=0.0, scale=-1.0)

            # recurrent: t2 = (1 + pa) * (Wrec' @ eT); u = -decay'*h + bias'
            def evac4(m, acc):
                nc.vector.scalar_tensor_tensor(
                    t2[:, m, :], pa[:, m, :], 1.0, acc[:], op0=OP.add, op1=OP.mult
                )
                nc.gpsimd.tensor_scalar(
                    u[:, m, :],
                    ht[:, m, :],
                    col(V_NDEC, m),
                    col(V_BIASP, m),
                    op0=OP.mult,
                    op1=OP.add,
                )

            mm_layer(wr_sb, et, evac4)

            # halves keep the tail latency down: reduce matmuls for half 0
            # can start while half 1 is still in the vector pipe
            dh_bf = rpool.tile([P, KC, NT], bf16, tag="dh_bf")
            nsplit = 4 if i == NB - 1 else 2  # short tail for the last tile
            step = KC // nsplit
            for s in [slice(j * step, (j + 1) * step) for j in range(nsplit)]:
                nc.vector.tensor_add(num[:, s, :], t2[:, s, :], u[:, s, :])
                # dh = num * (1 + exp(-pa))
                nc.vector.scalar_tensor_tensor(
                    dh[:, s, :], ex[:, s, :], 1.0, num[:, s, :],
                    op0=OP.add, op1=OP.mult,
                )
                nc.scalar.square(sq[:, s, :], dh[:, s, :])
                nc.scalar.copy(dh_bf[:, s, :], dh[:, s, :])
            state[i] = (dh, dh_bf, sq)

        def reduce_phase(i):
            # partition-axis sums via ones-matmuls over all H=1024
            dh, dh_bf, sq = state[i]
            s_ps = psS.tile([1, NT], f32, tag="sum")
            q_ps = psS.tile([1, NT], f32, tag="sumsq")
            for m in range(KC):
                nc.tensor.matmul(
                    s_ps[:], ones_col[:], dh_bf[:, m, :],
                    start=(m == 0), stop=(m == KC - 1),
                )
            for m in range(KC):
                nc.tensor.matmul(
                    q_ps[:], ones_col[:], sq[:, m, :],
                    start=(m == 0), stop=(m == KC - 1),
                )
            i32 = mybir.dt.int32
            mu_n = rowpool.tile([1, NT], f32, tag="mu_n")
            ms = rowpool.tile([1, NT], f32, tag="ms")
            musq = rowpool.tile([1, NT], f32, tag="musq")
            ve = rowpool.tile([1, NT], f32, tag="ve")
            vh = rowpool.tile([1, NT], f32, tag="vh")
            ya = rowpool.tile([1, NT], f32, tag="ya")
            yb = rowpool.tile([1, NT], f32, tag="yb")
            t1 = rowpool.tile([1, NT], f32, tag="t1")
            t2r = rowpool.tile([1, NT], f32, tag="t2r")
            dq = rowpool.tile([1, 2 * NT], f32, tag="dq")
            # PSUM reads must stay on the vector engine (gpsimd can't); the
            # rest of the tiny row math runs on the otherwise-idle gpsimd so
            # it never delays the in-order vector queue
            nc.vector.tensor_scalar_mul(mu_n[:], s_ps[:], -1.0 / H)
            nc.vector.tensor_scalar_mul(ms[:], q_ps[:], 1.0 / H)
            nc.gpsimd.tensor_mul(musq[:], mu_n[:], mu_n[:])
            # ve = var + eps = ms - musq + eps
            nc.gpsimd.tensor_sub(ve[:], ms[:], musq[:])
            nc.gpsimd.tensor_scalar_add(ve[:], ve[:], EPS)
            # rstd = rsqrt(ve): Quake initial guess + 2 Newton steps —
            # avoids ln/sqrt ACT funcs so the whole kernel stays in one
            # activation-table set (no table reloads)
            nc.gpsimd.tensor_scalar_mul(vh[:], ve[:], 0.5)
            nc.vector.tensor_scalar(
                ya[:].bitcast(i32), ve[:].bitcast(i32), 1, None,
                op0=OP.arith_shift_right,
            )
            nc.vector.tensor_scalar(
                yb[:].bitcast(i32), ya[:].bitcast(i32), -1, 0x5F3759DF,
                op0=OP.mult, op1=OP.add,
            )
            for src, dst in ((yb, ya), (ya, dq[:, 0:NT])):
                nc.gpsimd.tensor_mul(t1[:], src[:], src[:])
                nc.gpsimd.tensor_mul(t2r[:], t1[:], vh[:])
                nc.gpsimd.tensor_scalar(t2r[:], t2r[:], -1.0, 1.5,
                                        op0=OP.mult, op1=OP.add)
                nc.gpsimd.tensor_mul(dst[:], src[:], t2r[:])
            nc.gpsimd.tensor_mul(dq[:, NT:], mu_n[:], dq[:, 0:NT])
            state[i] = (dh, dq)

        def bcast_phase(i):
            dh, dq = state[i]
            pq = psB.tile([P, 2 * NT], f32, tag="PQ")
            nc.tensor.matmul(pq[:], ones_row[:], dq[:], start=True, stop=True)
            state[i] = (dh, pq)

        def epilogue_phase(i):
            dh, pq = state[i]
            outf = outpool.tile([P, KC, NT], f32, tag="outf")
            for m in range(KC):
                # out = lnw2*(dh*D + qn) (+ lnb) = ln_w*rstd*(dh-mu) + ln_b
                s1 = ypool.tile([P, NT], f32, tag="s1")
                nc.vector.scalar_tensor_tensor(
                    s1[:], dh[:, m, :], col(V_LNW2, m), pq[:, 0:NT],
                    op0=OP.mult, op1=OP.mult,
                )
                if lnb_zero:
                    nc.vector.scalar_tensor_tensor(
                        outf[:, m, :], pq[:, NT:], col(V_LNW2, m), s1[:],
                        op0=OP.mult, op1=OP.add,
                    )
                else:
                    s2 = ypool.tile([P, NT], f32, tag="s2")
                    nc.vector.scalar_tensor_tensor(
                        s2[:], pq[:, NT:], col(V_LNW2, m), s1[:],
                        op0=OP.mult, op1=OP.add,
                    )
                    nc.vector.tensor_scalar_add(outf[:, m, :], s2[:], col(V_LNB, m))
                if m % 2 == 1:  # stream results out as they complete
                    nc.sync.dma_start(
                        out=out_r[:, m - 1 : m + 1, i * NT : (i + 1) * NT],
                        in_=outf[:, m - 1 : m + 1, :],
                    )
            state[i] = None

        for i in range(NB):
            matmul_phase(
                i,
                (lambda j=i: reduce_phase(j - 1)) if i > 0 else (lambda: None),
                (lambda j=i: bcast_phase(j - 1)) if i > 0 else (lambda: None),
            )
            if i > 0:
                epilogue_phase(i - 1)
        reduce_phase(NB - 1)
        bcast_phase(NB - 1)
        epilogue_phase(NB - 1)

    if not nc.is_finalized():
        nc.finalize()
    return nc


def _get_nc(lnb_zero):
    key = ("nc", lnb_zero)
    if key not in _CACHED:
        _CACHED[key] = _build_nc(lnb_zero)
    return _CACHED[key]


# test.py can flip these before calling kernel() to profile
TRACE = False
LAST_RESULT = {}


def kernel(t, h, e, W_rec, bias, tau, decay, ln_w, ln_b,
           ce_w1, ce_b1, ce_w2, ce_b2, pm_w, pm_b):
    from concourse.bass_utils import run_bass_kernel_spmd

    f = np.float32
    h = np.asarray(h, f)
    e = np.asarray(e, f)
    W_rec = np.asarray(W_rec, f)
    bias = np.asarray(bias, f)
    tau = np.asarray(tau, f)
    decay = np.asarray(decay, f)
    ln_w = np.asarray(ln_w, f)
    ln_b = np.asarray(ln_b, f)
    ce_w1 = np.asarray(ce_w1, f)
    ce_b1 = np.asarray(ce_b1, f)
    ce_w2 = np.asarray(ce_w2, f)
    ce_b2 = np.asarray(ce_b2, f)
    pm_w = np.asarray(pm_w, f)
    pm_b = np.asarray(pm_b, f)

    invtau = 1.0 / tau
    negdecay = -decay * invtau
    biasp = bias * invtau
    pmb_eff = pm_b + pm_w @ ce_b2  # fold ce_b2 through the param modulator
    lnb_zero = bool(np.all(ln_b == 0.0))

    w1T = np.ascontiguousarray(ce_w1.T).astype(BF16)
    w2T = np.ascontiguousarray(ce_w2.T).astype(BF16)
    pmT = np.ascontiguousarray(pm_w.T).astype(BF16)
    wrT = np.ascontiguousarray(W_rec.T * invtau[None, :]).astype(BF16)

    def chunked(v):  # [H] -> [128, KC] with column m = chunk m
        return np.ascontiguousarray(v.reshape(KC, P).T)

    consts = np.concatenate(
        [chunked(v) for v in (ce_b1, pmb_eff, negdecay, biasp, ln_w, ln_b)],
        axis=1,
    ).astype(f)

    in_maps = []
    for i in range(NCORES):
        rows = slice(i * BL, (i + 1) * BL)
        in_maps.append({
            "hT": np.ascontiguousarray(h[rows].T).astype(BF16),
            "eT": np.ascontiguousarray(e[rows].T).astype(BF16),
            "w1T": w1T, "w2T": w2T, "pmT": pmT, "wrT": wrT,
            "consts": consts,
        })

    nc = _get_nc(lnb_zero)
    res = run_bass_kernel_spmd(nc, in_maps, core_ids=list(range(NCORES)),
                               trace=TRACE)
    LAST_RESULT["exec_time_ns"] = res.exec_time_ns
    LAST_RESULT["mean_exec_time_ns"] = res.mean_exec_time_ns
    LAST_RESULT["instructions_and_trace"] = res.instructions_and_trace

    out = np.empty((B, H), f)
    for i in range(NCORES):
        out[i * BL : (i + 1) * BL] = res.results[i]["out"].T
    return out
